# revision 2
# baseline (speedup 1.0000x reference)
"""Trainium2 Bass kernel for the Laplace-kernel feature expansion.

Reference computation (per scalar x of the [16, 64, 64, 64] input):
    phi_i  = exp(-|x - p_i|)            for 15 design points p_i
    out_j  = sum_i chol_inv[i, j] phi_i
scattered so out[b, c*15 + j, h, w] comes from x[b, c, h, w].

Distribution: pure data parallel, 2 batches per core across 8 cores.

Per-core dataflow (no collectives), engineered around the TRN2 engine cost
model (DVE/ScalarE PSUM-tier ops run at 1 elem/lane/cycle):
  - The PE array is split into four disjoint 32-strip rectangles via
    tile_position so the broadcast matmuls (replicate x onto 15 rows per
    channel and subtract p_i via a ones-row) and the block-diag chol_inv
    projection matmuls stream concurrently with no stationary-weight
    conflicts:
      bcastB rows 0-8   x cols 0-63   (K=9,  M=64) -> T[0:64]
      projB  rows 0-59  x cols 64-127 (K=60, M=64) -> O[64:128]
      bcastA rows 64-72 x cols 64-127 (K=9,  M=64) -> T[64:128]
      projA  rows 64-123x cols 0-63   (K=60, M=64) -> O[0:64]
    Each half handles 4 channels (hi/lo bf16 split of x + ones row = K=9).
  - Elementwise chain per [128, 1024] chunk (64 chunks/core):
      abs  : DVE int32 sign-clear in PSUM (in place)
      exp  : ScalarE Exp(scale=-1) PSUM->SBUF bf16
      evict: PSUM fp32 -> SBUF bf16, split DVE/ScalarE to balance engines
  - Output DMA'd per chunk as two [60, 1024] bf16 transfers.
"""

import sys

if "/opt/trn_rl_repo" not in sys.path:
    sys.path.insert(0, "/opt/trn_rl_repo")

import numpy as np
import ml_dtypes


def _ensure_axon_hooks_stub():
    """run_bass_kernel_spmd imports antenv.axon_hooks when BASS_TRACE is
    set; the module is absent on some images. Provide a no-op stub so a
    stray BASS_TRACE env var cannot crash the kernel (tracing is then
    skipped gracefully)."""
    try:
        import antenv.axon_hooks  # noqa: F401
    except ImportError:
        import types

        try:
            import antenv
        except ImportError:
            return
        mod = types.ModuleType("antenv.axon_hooks")
        _hook = [None]
        mod.set_axon_ntff_profile_hook = lambda h: _hook.__setitem__(0, h)
        mod.get_axon_ntff_profile_hook = lambda: _hook[0]
        sys.modules["antenv.axon_hooks"] = mod
        antenv.axon_hooks = mod


_ensure_axon_hooks_stub()

BF16 = ml_dtypes.bfloat16

B, C, H, W = 16, 64, 64, 64
P = H * W                # 4096 spatial positions
M_PTS = 15               # design points
CH_HALF = 4              # channels per PE half
MH = CH_HALF * M_PTS     # 60 live rows per half
KIN = 2 * CH_HALF + 1    # 9 moving rows per half (hi/lo pairs + ones)
NCORES = 8
BPC = B // NCORES        # batches per core (2)
CBLK = C // (2 * CH_HALF)  # 8-channel blocks per batch (8)
NCOLS = BPC * CBLK * P   # 65536 columns per core per half
CHUNK = 1024             # columns per elementwise chunk
NCHUNK = NCOLS // CHUNK  # 64 chunks per core

# Of the 64 chunk evictions per core, how many go to ScalarE (the rest
# go to VectorE). Engine balance: ScalarE also does all 64 exps, VectorE
# all 64 abs ops.
SCE_EVICT_NUM = 40

_CACHED = {}


def _build_nc():
    from concourse import bacc
    import concourse.mybir as mybir
    from concourse.tile import TileContext

    dt = mybir.dt
    Act = mybir.ActivationFunctionType
    Alu = mybir.AluOpType

    nc = bacc.Bacc(
        "TRN2", target_bir_lowering=False, debug=False, num_devices=NCORES
    )
    xa_d = nc.declare_dram_parameter("xa", [KIN, NCOLS], dt.bfloat16, isOutput=False)
    xb_d = nc.declare_dram_parameter("xb", [KIN, NCOLS], dt.bfloat16, isOutput=False)
    wa_d = nc.declare_dram_parameter("wa", [KIN, 64], dt.bfloat16, isOutput=False)
    wb_d = nc.declare_dram_parameter("wb", [KIN, 64], dt.bfloat16, isOutput=False)
    ra_d = nc.declare_dram_parameter("ra", [MH, 64], dt.bfloat16, isOutput=False)
    rb_d = nc.declare_dram_parameter("rb", [MH, 64], dt.bfloat16, isOutput=False)
    out_d = nc.declare_dram_parameter(
        "out", [BPC, 2 * CH_HALF * M_PTS * CBLK, P], dt.bfloat16, isOutput=True
    )

    with TileContext(nc) as tc:
        with (
            tc.tile_pool(name="xsb", bufs=1) as xpool,
            tc.tile_pool(name="wsb", bufs=1) as wpool,
            tc.tile_pool(name="phi", bufs=3) as ppool,
            tc.tile_pool(name="ost", bufs=3) as opool,
            tc.tile_pool(name="psT", bufs=2, space="PSUM") as psT,
            tc.tile_pool(name="psO", bufs=2, space="PSUM") as psO,
        ):
            # x for half B at partitions 0-8, half A at partitions 64-72.
            xs = xpool.tile([128, NCOLS], dt.bfloat16)
            ws = wpool.tile([128, 64], dt.bfloat16)
            rs = wpool.tile([128, 64], dt.bfloat16)
            # Graduated loads: small leading slices so chunk 0 can start
            # while the bulk streams in behind it.
            nc.sync.dma_start(out=xs[0:KIN, 0:1024], in_=xb_d[:, 0:1024])
            nc.sync.dma_start(out=ws[0:KIN, :], in_=wb_d[:, :])
            nc.sync.dma_start(out=xs[64 : 64 + KIN, 0:1024], in_=xa_d[:, 0:1024])
            nc.sync.dma_start(out=ws[64 : 64 + KIN, :], in_=wa_d[:, :])
            nc.sync.dma_start(out=rs[0:MH, :], in_=rb_d[:, :])
            nc.sync.dma_start(out=rs[64 : 64 + MH, :], in_=ra_d[:, :])
            pos = 1024
            for width in (3072, 12288, 49152):
                nc.sync.dma_start(
                    out=xs[0:KIN, pos : pos + width], in_=xb_d[:, pos : pos + width]
                )
                nc.sync.dma_start(
                    out=xs[64 : 64 + KIN, pos : pos + width],
                    in_=xa_d[:, pos : pos + width],
                )
                pos += width

            sce_acc = 0
            for t in range(NCHUNK):
                b, rem = divmod(t, CBLK * (P // CHUNK))
                cb, q = divmod(rem, P // CHUNK)
                c0 = t * CHUNK

                T = psT.tile([128, CHUNK], dt.float32)
                for l in range(2):
                    sl = slice(l * 512, (l + 1) * 512)
                    xsl = slice(c0 + l * 512, c0 + (l + 1) * 512)
                    nc.tensor.matmul(
                        T[0:64, sl], ws[0:KIN, :], xs[0:KIN, xsl],
                        start=True, stop=True, tile_position=(0, 0),
                    )
                    nc.tensor.matmul(
                        T[64:128, sl], ws[64 : 64 + KIN, :], xs[64 : 64 + KIN, xsl],
                        start=True, stop=True, tile_position=(64, 64),
                    )
                # |T| via sign-bit clear on an int32 view, in place in PSUM
                nc.vector.tensor_scalar(
                    out=T[:, :].bitcast(dt.int32),
                    in0=T[:, :].bitcast(dt.int32),
                    scalar1=0x7FFFFFFF, scalar2=None, op0=Alu.bitwise_and,
                )
                phi = ppool.tile([128, CHUNK], dt.bfloat16)
                nc.scalar.activation(phi[:, :], T[:, :], Act.Exp, scale=-1.0)

                O = psO.tile([128, CHUNK], dt.float32)
                for l in range(2):
                    sl = slice(l * 512, (l + 1) * 512)
                    nc.tensor.matmul(
                        O[0:64, sl], rs[64 : 64 + MH, :], phi[64 : 64 + MH, sl],
                        start=True, stop=True, tile_position=(64, 0),
                    )
                    nc.tensor.matmul(
                        O[64:128, sl], rs[0:MH, :], phi[0:MH, sl],
                        start=True, stop=True, tile_position=(0, 64),
                    )
                ost = opool.tile([128, CHUNK], dt.bfloat16)
                sce_acc += SCE_EVICT_NUM
                if sce_acc >= NCHUNK:
                    sce_acc -= NCHUNK
                    nc.scalar.activation(ost[:, :], O[:, :], Act.Copy)
                else:
                    nc.vector.tensor_copy(out=ost[:, :], in_=O[:, :])
                row0 = 120 * cb
                csl = slice(q * CHUNK, (q + 1) * CHUNK)
                nc.sync.dma_start(
                    out=out_d[b, row0 : row0 + MH, csl], in_=ost[0:MH, :]
                )
                nc.sync.dma_start(
                    out=out_d[b, row0 + MH : row0 + 2 * MH, csl],
                    in_=ost[64 : 64 + MH, :],
                )
    nc.compile()
    return nc


def _host_prep(x, design_points, chol_inv):
    """Build the derived host-side arrays fed to the device."""
    pts = np.asarray(design_points, dtype=np.float32)
    xs = np.ascontiguousarray(np.asarray(x, dtype=np.float32)).reshape(B, C, P)
    x_hi = xs.astype(BF16)
    x_lo = (xs - x_hi.astype(np.float32)).astype(BF16)

    # Row layout per half: [c0_hi, c0_lo, c1_hi, c1_lo, c2_hi, c2_lo,
    # c3_hi, c3_lo, ones]; half A = channels 0-3 of each 8-block, half B
    # = channels 4-7. Columns: (b, cblk, spatial).
    def rows(a, lo_of):  # a [B, C, P] -> [2(half), KIN, B, CBLK, P]
        r = np.empty((2, KIN, B, CBLK, P), dtype=BF16)
        a5 = a.reshape(B, CBLK, 2, CH_HALF, P)  # [b, cb, half, c, p]
        l5 = lo_of.reshape(B, CBLK, 2, CH_HALF, P)
        for h in range(2):
            r[h, 0 : 2 * CH_HALF : 2] = a5[:, :, h].transpose(2, 0, 1, 3)
            r[h, 1 : 2 * CH_HALF : 2] = l5[:, :, h].transpose(2, 0, 1, 3)
        r[:, 2 * CH_HALF] = BF16(1.0)
        return r

    arr = rows(x_hi, x_lo)  # [2, KIN, B, CBLK, P]

    w = np.zeros((2, KIN, 64), dtype=np.float32)
    for c in range(CH_HALF):
        w[:, 2 * c, 15 * c : 15 * c + 15] = 1.0
        w[:, 2 * c + 1, 15 * c : 15 * c + 15] = 1.0
        w[:, 2 * CH_HALF, 15 * c : 15 * c + 15] = -pts
    w = w.astype(BF16)

    chol = np.asarray(chol_inv, dtype=np.float32)
    r_blk = np.zeros((MH, 64), dtype=np.float32)
    for c in range(CH_HALF):
        r_blk[15 * c : 15 * c + 15, 15 * c : 15 * c + 15] = chol
    r_blk = r_blk.astype(BF16)

    return arr, w, r_blk


LAST_RESULT = None


def kernel(x, design_points, chol_inv):
    global LAST_RESULT
    from concourse.bass_utils import run_bass_kernel_spmd

    if "nc" not in _CACHED:
        _CACHED["nc"] = _build_nc()
    nc = _CACHED["nc"]

    arr, w, r_blk = _host_prep(x, design_points, chol_inv)

    in_maps = []
    for core in range(NCORES):
        sl = arr[:, :, core * BPC : (core + 1) * BPC]  # [2, KIN, BPC, CBLK, P]
        in_maps.append(
            {
                "xa": np.ascontiguousarray(sl[0].reshape(KIN, NCOLS)),
                "xb": np.ascontiguousarray(sl[1].reshape(KIN, NCOLS)),
                "wa": w[0],
                "wb": w[1],
                "ra": r_blk,
                "rb": r_blk,
            }
        )

    res = run_bass_kernel_spmd(nc, in_maps, core_ids=list(range(NCORES)))
    LAST_RESULT = res

    full = np.empty((B, C * M_PTS, P), dtype=np.float32)
    for core in range(NCORES):
        full[core * BPC : (core + 1) * BPC] = res.results[core]["out"]
    return full.reshape(B, C * M_PTS, H, W)


# revision 7
# speedup vs baseline: 1.4013x; 1.4013x over previous
"""Trainium2 Bass kernel for the Laplace-kernel feature expansion.

Reference computation (per scalar x of the [16, 64, 64, 64] input):
    phi_i  = exp(-|x - p_i|)            for 15 design points p_i
    out_j  = sum_i chol_inv[i, j] phi_i
scattered so out[b, c*15 + j, h, w] comes from x[b, c, h, w].

Distribution: pure data parallel, 2 batches per core across 8 cores.

Per-core dataflow (no collectives):
  1. x is pre-split on host into bf16 (hi, lo) pairs, laid out so one
     [128, 16384] DMA (32 KB contiguous per partition, all 16 DMA
     engines) loads the whole per-core input into SBUF once.
  2. TensorE "broadcast" matmuls with a 0/1 block matrix replicate each
     x value onto 15 partitions (8 channel groups x 15 = 120 partitions),
     reconstructing fp32 x = hi + lo in PSUM; an extra ones-row makes the
     same matmul subtract the design point p_i (p_i exact in bf16).
     The K=17 matmuls are packed 4x into the 128x128 array via
     tile_position row-tiling (4 concurrent quadrant matmuls).
  3. VectorE computes |T| in one op (sign-bit clear on an int32 view).
  4. ScalarE computes exp(-|t|) -> bf16.
  5. TensorE applies block-diag(chol_inv) -> PSUM (fp32).
  6. PSUM evicted to SBUF (split between ScalarE/VectorE), DMA to DRAM.

Spatial mapping: PE-array quadrant q = 2j+l covers, within a (b, cblock)
tile, the spatial columns 2048j + 1024h + 512l + c (h = half), so each
post-projection PSUM chunk evicts to a contiguous 1024-column span.
"""

import sys

if "/opt/trn_rl_repo" not in sys.path:
    sys.path.insert(0, "/opt/trn_rl_repo")

import numpy as np
import ml_dtypes


def _ensure_axon_hooks_stub():
    """run_bass_kernel_spmd imports antenv.axon_hooks when BASS_TRACE is
    set; the module is absent on some images. Provide a no-op stub so a
    stray BASS_TRACE env var cannot crash the kernel (tracing is then
    skipped gracefully)."""
    try:
        import antenv.axon_hooks  # noqa: F401
    except ImportError:
        import types

        try:
            import antenv
        except ImportError:
            return
        mod = types.ModuleType("antenv.axon_hooks")
        _hook = [None]
        mod.set_axon_ntff_profile_hook = lambda h: _hook.__setitem__(0, h)
        mod.get_axon_ntff_profile_hook = lambda: _hook[0]
        sys.modules["antenv.axon_hooks"] = mod
        antenv.axon_hooks = mod


_ensure_axon_hooks_stub()

BF16 = ml_dtypes.bfloat16

B, C, H, W = 16, 64, 64, 64
P = H * W                # 4096 spatial positions
M_PTS = 15               # design points
G = 8                    # channel groups per tile
MROWS = G * M_PTS        # 120 partitions used
KIN = 2 * G + 1          # 17 moving rows for the broadcast matmul
NCORES = 8
BPC = B // NCORES        # batches per core (2)
CBLK = C // G            # channel-block tiles per batch (8)
NTILES = BPC * CBLK      # 16 (b, cblock) tiles per core
QCOLS = NTILES * 1024    # 16384 columns per quadrant row

# Engine balance (measured cost model): ScalarE does all 64 exps plus
# SCE_ABS_NUM of the 64 abs ops (ScalarE Abs 997ns vs DVE 1222ns per
# [*,1024] chunk); VectorE does the remaining abs ops plus all 128
# [*,512] evictions (DVE cast 683ns vs ScalarE copy 804ns).
DVE_EVICT_NUM = 128
TOTAL_EVICTS = 128
SCE_ABS_NUM = 46
TOTAL_ABS = 64
# Dependency-free matmuls issued while the input DMA streams in, so the
# PE's HAM activity window sees a continuous busy burst and un-throttles
# the clock gate (K=4/8 -> 8/8) before the real pipeline starts.
WARMUP_MMS = 24

_CACHED = {}


def _build_nc():
    from concourse import bacc
    import concourse.mybir as mybir
    from concourse.tile import TileContext

    dt = mybir.dt
    Act = mybir.ActivationFunctionType
    Alu = mybir.AluOpType

    nc = bacc.Bacc(
        "TRN2", target_bir_lowering=False, debug=False, num_devices=NCORES
    )
    x_full = nc.declare_dram_parameter(
        "x_full", [128, QCOLS], dt.bfloat16, isOutput=False
    )
    w4 = nc.declare_dram_parameter("w4", [128, 128], dt.bfloat16, isOutput=False)
    r_blk = nc.declare_dram_parameter(
        "r_blk", [MROWS, 128], dt.bfloat16, isOutput=False
    )
    out = nc.declare_dram_parameter(
        "out", [BPC, C * M_PTS, P], dt.bfloat16, isOutput=True
    )

    with TileContext(nc) as tc:
        with (
            tc.tile_pool(name="const", bufs=1) as cpool,
            tc.tile_pool(name="xbig", bufs=1) as xpool,
            tc.tile_pool(name="absT", bufs=4) as apool,
            tc.tile_pool(name="phi", bufs=4) as ppool,
            tc.tile_pool(name="osb", bufs=4) as opool,
            tc.tile_pool(name="psT", bufs=1, space="PSUM") as psTp,
            tc.tile_pool(name="psO", bufs=2, space="PSUM") as psOp,
        ):
            # Whole per-core input resident in SBUF (32 KB/partition),
            # graduated full-width DMAs so all 16 DMA engines participate
            # and the first tile's data (plus weights) arrives quickly.
            xbig = xpool.tile([128, QCOLS], dt.bfloat16)
            nc.sync.dma_start(out=xbig[:, 0:512], in_=x_full[:, 0:512])
            w4_t = cpool.tile([128, 128], dt.bfloat16)
            nc.sync.dma_start(out=w4_t[:], in_=w4[:, :])
            nc.sync.dma_start(out=xbig[:, 512:1024], in_=x_full[:, 512:1024])
            r_t = cpool.tile([MROWS, 128], dt.bfloat16)
            nc.sync.dma_start(out=r_t[:], in_=r_blk[:, :])
            pos = 1024
            for ntile_chunk in (1, 2, 4, 4, 4):
                w = ntile_chunk * 1024
                nc.sync.dma_start(
                    out=xbig[:, pos : pos + w], in_=x_full[:, pos : pos + w]
                )
                pos += w

            # HAM warm-up: back-to-back matmuls into a scratch PSUM tile
            # while the bulk of x is still streaming in.
            warm = psTp.tile([128, 1024], dt.float32, name="tps0", tag="tps0")
            for _ in range(WARMUP_MMS):
                nc.tensor.matmul(
                    warm[:, 0:512], w4_t[0:KIN, :], xbig[0:KIN, 0:512],
                    start=True, stop=True,
                )

            gc = 0
            ac = 0
            sce_abs_acc = 0
            tcnt = 0
            for t in range(NTILES):
                b, cb = divmod(t, CBLK)
                ot = opool.tile([MROWS, P], dt.bfloat16)
                for h in range(2):
                    tchunks = [
                        psTp.tile(
                            [128, 1024],
                            dt.float32,
                            name=f"tps{(tcnt + j) % 3}",
                            tag=f"tps{(tcnt + j) % 3}",
                        )
                        for j in range(2)
                    ]
                    tcnt += 2
                    # 4 concurrent quadrant matmuls (row-tiled PE array)
                    for q in range(4):
                        j, l = divmod(q, 2)
                        nc.tensor.matmul(
                            tchunks[j][:, l * 512 : (l + 1) * 512],
                            w4_t[32 * q : 32 * q + KIN, :],
                            xbig[
                                32 * q : 32 * q + KIN,
                                t * 1024 + h * 512 : t * 1024 + (h + 1) * 512,
                            ],
                            start=True,
                            stop=True,
                            tile_position=(32 * q, 0),
                        )
                    # both abs ops back-to-back on DVE; |T| computed
                    # in place in PSUM so exp reads via ScalarE's faster
                    # PSUM port and no SBUF intermediate is needed
                    pts = []
                    for j in range(2):
                        tps = tchunks[j]
                        sce_abs_acc += SCE_ABS_NUM
                        if sce_abs_acc >= TOTAL_ABS:
                            # |T| in place on ScalarE (Abs activation)
                            sce_abs_acc -= TOTAL_ABS
                            nc.scalar.activation(
                                tps[0:MROWS, :], tps[0:MROWS, :], Act.Abs
                            )
                        else:
                            # |T| via sign-bit clear on an int32 view (DVE)
                            nc.vector.tensor_scalar(
                                out=tps[0:MROWS, :].bitcast(dt.int32),
                                in0=tps[0:MROWS, :].bitcast(dt.int32),
                                scalar1=0x7FFFFFFF,
                                scalar2=None,
                                op0=Alu.bitwise_and,
                            )
                        ac += 1
                        pt = ppool.tile([MROWS, 1024], dt.bfloat16, name=f"pt{j}")
                        nc.scalar.activation(
                            pt[:], tps[0:MROWS, :], Act.Exp, scale=-1.0
                        )
                        pts.append(pt)
                    for j in range(2):
                        pt = pts[j]
                        for l in range(2):
                            ops = psOp.tile([128, 512], dt.float32)
                            nc.tensor.matmul(
                                ops[:],
                                r_t[:],
                                pt[:, l * 512 : (l + 1) * 512],
                                start=True,
                                stop=True,
                            )
                            base = 2048 * j + 1024 * h + 512 * l
                            dst = ot[:, base : base + 512]
                            if (gc * DVE_EVICT_NUM) % TOTAL_EVICTS < DVE_EVICT_NUM:
                                nc.vector.tensor_copy(out=dst, in_=ops[0:MROWS, :])
                            else:
                                nc.scalar.activation(dst, ops[0:MROWS, :], Act.Copy)
                            gc += 1
                nc.sync.dma_start(
                    out=out[b, cb * MROWS : (cb + 1) * MROWS, :], in_=ot[:]
                )
    nc.compile()
    return nc


def _host_prep(x, design_points, chol_inv):
    """Build the derived host-side arrays fed to the device."""
    pts = np.asarray(design_points, dtype=np.float32)
    xs = np.ascontiguousarray(np.asarray(x, dtype=np.float32)).reshape(B, C, P)
    x_hi = xs.astype(BF16)
    x_lo = (xs - x_hi.astype(np.float32)).astype(BF16)

    # spatial = 2048j + 1024h + 512l + c ; quadrant q = 2j + l
    # arr[q, r, b, cb, h, c(512)] with r = 2g + part (hi/lo), r=16 -> 1.0
    def to_quad(a):  # [B, C, P] -> [4(q), G, B, CBLK, 2(h), 512]
        a7 = a.reshape(B, CBLK, G, 2, 2, 2, 512)  # [b, cb, g, j, h, l, c]
        return a7.transpose(3, 5, 2, 0, 1, 4, 6).reshape(4, G, B, CBLK, 2, 512)

    arr = np.empty((4, KIN, B, CBLK, 2, 512), dtype=BF16)
    arr[:, 0 : 2 * G : 2] = to_quad(x_hi)
    arr[:, 1 : 2 * G : 2] = to_quad(x_lo)
    arr[:, 2 * G] = BF16(1.0)

    w17 = np.zeros((KIN, 128), dtype=np.float32)
    for g in range(G):
        w17[2 * g, 15 * g : 15 * g + 15] = 1.0
        w17[2 * g + 1, 15 * g : 15 * g + 15] = 1.0
        w17[2 * G, 15 * g : 15 * g + 15] = -pts
    w4 = np.zeros((128, 128), dtype=np.float32)
    for q in range(4):
        w4[32 * q : 32 * q + KIN] = w17
    w4 = w4.astype(BF16)

    chol = np.asarray(chol_inv, dtype=np.float32)
    r_blk = np.zeros((MROWS, 128), dtype=np.float32)
    for g in range(G):
        r_blk[15 * g : 15 * g + 15, 15 * g : 15 * g + 15] = chol
    r_blk = r_blk.astype(BF16)

    return arr, w4, r_blk


LAST_RESULT = None


def kernel(x, design_points, chol_inv):
    global LAST_RESULT
    from concourse.bass_utils import run_bass_kernel_spmd

    if "nc" not in _CACHED:
        _CACHED["nc"] = _build_nc()
    nc = _CACHED["nc"]

    arr, w4, r_blk = _host_prep(x, design_points, chol_inv)

    in_maps = []
    for core in range(NCORES):
        # per-core [4, 17, 16384] placed into a [128, 16384] buffer at
        # partition offsets 32q (rows 17..31 of each quadrant unused)
        x_q = arr[:, :, core * BPC : (core + 1) * BPC].reshape(4, KIN, QCOLS)
        x_full = np.zeros((128, QCOLS), dtype=BF16)
        for q in range(4):
            x_full[32 * q : 32 * q + KIN] = x_q[q]
        in_maps.append({"x_full": x_full, "w4": w4, "r_blk": r_blk})

    res = run_bass_kernel_spmd(nc, in_maps, core_ids=list(range(NCORES)))
    LAST_RESULT = res

    full = np.empty((B, C * M_PTS, P), dtype=np.float32)
    for core in range(NCORES):
        full[core * BPC : (core + 1) * BPC] = res.results[core]["out"]
    return full.reshape(B, C * M_PTS, H, W)



# revision 9
# speedup vs baseline: 1.5526x; 1.1080x over previous
"""Trainium2 Bass kernel for the Laplace-kernel feature expansion.

Reference computation (per scalar x of the [16, 64, 64, 64] input):
    phi_i  = exp(-|x - p_i|)            for 15 design points p_i
    out_j  = sum_i chol_inv[i, j] phi_i
scattered so out[b, c*15 + j, h, w] comes from x[b, c, h, w].

Distribution: pure data parallel, 2 batches per core across 8 cores.

Per-core dataflow (no collectives):
  1. x is pre-split on host into bf16 (hi, lo) pairs, laid out so one
     [128, 16384] DMA (32 KB contiguous per partition, all 16 DMA
     engines) loads the whole per-core input into SBUF once.
  2. TensorE "broadcast" matmuls with a 0/1 block matrix replicate each
     x value onto 15 partitions (8 channel groups x 15 = 120 partitions),
     reconstructing fp32 x = hi + lo in PSUM; an extra ones-row makes the
     same matmul subtract the design point p_i (p_i exact in bf16).
     The K=17 matmuls are packed 4x into the 128x128 array via
     tile_position row-tiling (4 concurrent quadrant matmuls).
  3. VectorE computes |T| in one op (sign-bit clear on an int32 view).
  4. ScalarE computes exp(-|t|) -> bf16.
  5. TensorE applies block-diag(chol_inv) -> PSUM (fp32).
  6. PSUM evicted to SBUF (split between ScalarE/VectorE), DMA to DRAM.

Spatial mapping: PE-array quadrant q = 2j+l covers, within a (b, cblock)
tile, the spatial columns 2048j + 1024h + 512l + c (h = half), so each
post-projection PSUM chunk evicts to a contiguous 1024-column span.
"""

import sys

if "/opt/trn_rl_repo" not in sys.path:
    sys.path.insert(0, "/opt/trn_rl_repo")

import numpy as np
import ml_dtypes


def _ensure_axon_hooks_stub():
    """run_bass_kernel_spmd imports antenv.axon_hooks when BASS_TRACE is
    set; the module is absent on some images. Provide a no-op stub so a
    stray BASS_TRACE env var cannot crash the kernel (tracing is then
    skipped gracefully)."""
    try:
        import antenv.axon_hooks  # noqa: F401
    except ImportError:
        import types

        try:
            import antenv
        except ImportError:
            return
        mod = types.ModuleType("antenv.axon_hooks")
        _hook = [None]
        mod.set_axon_ntff_profile_hook = lambda h: _hook.__setitem__(0, h)
        mod.get_axon_ntff_profile_hook = lambda: _hook[0]
        sys.modules["antenv.axon_hooks"] = mod
        antenv.axon_hooks = mod


_ensure_axon_hooks_stub()

BF16 = ml_dtypes.bfloat16

B, C, H, W = 16, 64, 64, 64
P = H * W                # 4096 spatial positions
M_PTS = 15               # design points
G = 8                    # channel groups per tile
MROWS = G * M_PTS        # 120 partitions used
KIN = 2 * G + 1          # 17 moving rows for the broadcast matmul
NCORES = 8
BPC = B // NCORES        # batches per core (2)
CBLK = C // G            # channel-block tiles per batch (8)
NTILES = BPC * CBLK      # 16 (b, cblock) tiles per core
QCOLS = NTILES * 1024    # 16384 columns per quadrant row

# Engine balance (measured cost model): ScalarE does all 64 exps plus
# SCE_ABS_NUM of the 64 abs ops (ScalarE Abs 997ns vs DVE 1222ns per
# [*,1024] chunk); VectorE does the remaining abs ops plus all 128
# [*,512] evictions (DVE cast 683ns vs ScalarE copy 804ns).
DVE_EVICT_NUM = 57
TOTAL_EVICTS = 128
SCE_ABS_NUM = 0
TOTAL_ABS = 64
# Dependency-free matmuls issued while the input DMA streams in, so the
# PE's HAM activity window sees a continuous busy burst and un-throttles
# the clock gate (K=4/8 -> 8/8) before the real pipeline starts.
WARMUP_MMS = 24

_CACHED = {}


def _build_nc():
    from concourse import bacc
    import concourse.mybir as mybir
    from concourse.tile import TileContext

    dt = mybir.dt
    Act = mybir.ActivationFunctionType
    Alu = mybir.AluOpType

    nc = bacc.Bacc(
        "TRN2", target_bir_lowering=False, debug=False, num_devices=NCORES
    )
    x_full = nc.declare_dram_parameter(
        "x_full", [128, QCOLS], dt.bfloat16, isOutput=False
    )
    w4 = nc.declare_dram_parameter("w4", [128, 128], dt.bfloat16, isOutput=False)
    r_blk = nc.declare_dram_parameter(
        "r_blk", [MROWS, 128], dt.bfloat16, isOutput=False
    )
    out = nc.declare_dram_parameter(
        "out", [BPC, C * M_PTS, P], dt.bfloat16, isOutput=True
    )

    with TileContext(nc) as tc:
        with (
            tc.tile_pool(name="const", bufs=1) as cpool,
            tc.tile_pool(name="xbig", bufs=1) as xpool,
            tc.tile_pool(name="absT", bufs=4) as apool,
            tc.tile_pool(name="phi", bufs=4) as ppool,
            tc.tile_pool(name="osb", bufs=4) as opool,
            tc.tile_pool(name="psT", bufs=1, space="PSUM") as psTp,
            tc.tile_pool(name="psO", bufs=2, space="PSUM") as psOp,
        ):
            # Whole per-core input resident in SBUF (32 KB/partition),
            # graduated full-width DMAs so all 16 DMA engines participate
            # and the first tile's data (plus weights) arrives quickly.
            xbig = xpool.tile([128, QCOLS], dt.bfloat16)
            nc.sync.dma_start(out=xbig[:, 0:512], in_=x_full[:, 0:512])
            w4_t = cpool.tile([128, 128], dt.bfloat16)
            nc.sync.dma_start(out=w4_t[:], in_=w4[:, :])
            nc.sync.dma_start(out=xbig[:, 512:1024], in_=x_full[:, 512:1024])
            r_t = cpool.tile([MROWS, 128], dt.bfloat16)
            nc.sync.dma_start(out=r_t[:], in_=r_blk[:, :])
            pos = 1024
            for ntile_chunk in (1, 2, 4, 4, 4):
                w = ntile_chunk * 1024
                nc.sync.dma_start(
                    out=xbig[:, pos : pos + w], in_=x_full[:, pos : pos + w]
                )
                pos += w

            # HAM warm-up: back-to-back matmuls into a scratch PSUM tile
            # while the bulk of x is still streaming in.
            # Rotate across 4 PSUM banks: back-to-back matmuls into one
            # bank WAW-serialize fill/drain and HAM never sees the PE as
            # continuously busy, so it stays clock-gated at 1.2 GHz.
            warm = [
                psTp.tile([128, 1024], dt.float32, name=f"tps{i}", tag=f"tps{i}")
                for i in range(2)
            ]
            for i in range(WARMUP_MMS):
                wt = warm[(i // 2) % 2]
                sl = slice(512 * (i % 2), 512 * (i % 2) + 512)
                nc.tensor.matmul(
                    wt[:, sl], w4_t[0:KIN, :], xbig[0:KIN, 0:512],
                    start=True, stop=True,
                )

            gc = 0
            ac = 0
            sce_abs_acc = 0
            tcnt = 0
            for t in range(NTILES):
                b, cb = divmod(t, CBLK)
                ot = opool.tile([MROWS, P], dt.bfloat16)
                for h in range(2):
                    tchunks = [
                        psTp.tile(
                            [128, 1024],
                            dt.float32,
                            name=f"tps{(tcnt + j) % 3}",
                            tag=f"tps{(tcnt + j) % 3}",
                        )
                        for j in range(2)
                    ]
                    tcnt += 2
                    # 4 concurrent quadrant matmuls (row-tiled PE array)
                    for q in range(4):
                        j, l = divmod(q, 2)
                        nc.tensor.matmul(
                            tchunks[j][:, l * 512 : (l + 1) * 512],
                            w4_t[32 * q : 32 * q + KIN, :],
                            xbig[
                                32 * q : 32 * q + KIN,
                                t * 1024 + h * 512 : t * 1024 + (h + 1) * 512,
                            ],
                            start=True,
                            stop=True,
                            tile_position=(32 * q, 0),
                        )
                    # both abs ops back-to-back on DVE; |T| computed
                    # in place in PSUM so exp reads via ScalarE's faster
                    # PSUM port and no SBUF intermediate is needed
                    pts = []
                    for j in range(2):
                        tps = tchunks[j]
                        sce_abs_acc += SCE_ABS_NUM
                        if sce_abs_acc >= TOTAL_ABS:
                            # |T| in place on ScalarE (Abs activation)
                            sce_abs_acc -= TOTAL_ABS
                            nc.scalar.activation(
                                tps[0:MROWS, :], tps[0:MROWS, :], Act.Abs
                            )
                        else:
                            # |T| via sign-bit clear on an int32 view (DVE)
                            nc.vector.tensor_scalar(
                                out=tps[0:MROWS, :].bitcast(dt.int32),
                                in0=tps[0:MROWS, :].bitcast(dt.int32),
                                scalar1=0x7FFFFFFF,
                                scalar2=None,
                                op0=Alu.bitwise_and,
                            )
                        ac += 1
                        pt = ppool.tile([MROWS, 1024], dt.bfloat16, name=f"pt{j}")
                        nc.scalar.activation(
                            pt[:], tps[0:MROWS, :], Act.Exp, scale=-1.0
                        )
                        pts.append(pt)
                    for j in range(2):
                        pt = pts[j]
                        for l in range(2):
                            ops = psOp.tile([128, 512], dt.float32)
                            nc.tensor.matmul(
                                ops[:],
                                r_t[:],
                                pt[:, l * 512 : (l + 1) * 512],
                                start=True,
                                stop=True,
                            )
                            base = 2048 * j + 1024 * h + 512 * l
                            dst = ot[:, base : base + 512]
                            if (gc * DVE_EVICT_NUM) % TOTAL_EVICTS < DVE_EVICT_NUM:
                                nc.vector.tensor_copy(out=dst, in_=ops[0:MROWS, :])
                            else:
                                nc.scalar.activation(dst, ops[0:MROWS, :], Act.Copy)
                            gc += 1
                nc.sync.dma_start(
                    out=out[b, cb * MROWS : (cb + 1) * MROWS, :], in_=ot[:]
                )
    nc.compile()
    return nc


def _host_prep(x, design_points, chol_inv):
    """Build the derived host-side arrays fed to the device."""
    pts = np.asarray(design_points, dtype=np.float32)
    xs = np.ascontiguousarray(np.asarray(x, dtype=np.float32)).reshape(B, C, P)
    x_hi = xs.astype(BF16)
    x_lo = (xs - x_hi.astype(np.float32)).astype(BF16)

    # spatial = 2048j + 1024h + 512l + c ; quadrant q = 2j + l
    # arr[q, r, b, cb, h, c(512)] with r = 2g + part (hi/lo), r=16 -> 1.0
    def to_quad(a):  # [B, C, P] -> [4(q), G, B, CBLK, 2(h), 512]
        a7 = a.reshape(B, CBLK, G, 2, 2, 2, 512)  # [b, cb, g, j, h, l, c]
        return a7.transpose(3, 5, 2, 0, 1, 4, 6).reshape(4, G, B, CBLK, 2, 512)

    arr = np.empty((4, KIN, B, CBLK, 2, 512), dtype=BF16)
    arr[:, 0 : 2 * G : 2] = to_quad(x_hi)
    arr[:, 1 : 2 * G : 2] = to_quad(x_lo)
    arr[:, 2 * G] = BF16(1.0)

    w17 = np.zeros((KIN, 128), dtype=np.float32)
    for g in range(G):
        w17[2 * g, 15 * g : 15 * g + 15] = 1.0
        w17[2 * g + 1, 15 * g : 15 * g + 15] = 1.0
        w17[2 * G, 15 * g : 15 * g + 15] = -pts
    w4 = np.zeros((128, 128), dtype=np.float32)
    for q in range(4):
        w4[32 * q : 32 * q + KIN] = w17
    w4 = w4.astype(BF16)

    chol = np.asarray(chol_inv, dtype=np.float32)
    r_blk = np.zeros((MROWS, 128), dtype=np.float32)
    for g in range(G):
        r_blk[15 * g : 15 * g + 15, 15 * g : 15 * g + 15] = chol
    r_blk = r_blk.astype(BF16)

    return arr, w4, r_blk


LAST_RESULT = None


def kernel(x, design_points, chol_inv):
    global LAST_RESULT
    from concourse.bass_utils import run_bass_kernel_spmd

    if "nc" not in _CACHED:
        _CACHED["nc"] = _build_nc()
    nc = _CACHED["nc"]

    arr, w4, r_blk = _host_prep(x, design_points, chol_inv)

    in_maps = []
    for core in range(NCORES):
        # per-core [4, 17, 16384] placed into a [128, 16384] buffer at
        # partition offsets 32q (rows 17..31 of each quadrant unused)
        x_q = arr[:, :, core * BPC : (core + 1) * BPC].reshape(4, KIN, QCOLS)
        x_full = np.zeros((128, QCOLS), dtype=BF16)
        for q in range(4):
            x_full[32 * q : 32 * q + KIN] = x_q[q]
        in_maps.append({"x_full": x_full, "w4": w4, "r_blk": r_blk})

    res = run_bass_kernel_spmd(nc, in_maps, core_ids=list(range(NCORES)))
    LAST_RESULT = res

    full = np.empty((B, C * M_PTS, P), dtype=np.float32)
    for core in range(NCORES):
        full[core * BPC : (core + 1) * BPC] = res.results[core]["out"]
    return full.reshape(B, C * M_PTS, H, W)

